# revision 40
# baseline (speedup 1.0000x reference)
"""Trainium2 Bass kernel for a dense transformer block (B=4, T=1024, C=1024, H=16).

Sharding: 2 cores per batch element (8 cores / 4 batches). Each core computes
K/V (+LN1) for its full batch but only 4 of the 8 query blocks of 128 rows.
Query blocks are interleaved ({7,4,3,0} on even cores, {6,5,2,1} on odd) so the
causal-attention work is balanced; the compiled program is identical on every
core (SPMD) - per-core behaviour comes only from input data (x slice, gathered
query rows, causal-mask tiles).

v2 layout/scheduling:
- LN gamma/beta folded into the next matmul's weights+bias on the HOST
  (wq' = g*wq, bq' = bq + b@wq, ...), so the device LN is only (x-m)*rstd.
- h transposes done by the DMA XBAR (dma_start_transpose) instead of the PE.
- V projection is streamed per row-block right after that block's LN+transpose,
  K projection in two halves, so the tensor engine is densely busy from the
  start (keeps the HAM clock-gate at full rate).
- Attention scores for all 8 heads of a group land in one PSUM tile
  [128, 2, 4, 128] (hh-major so concurrent row-tiled matmuls hit different
  PSUM banks), exp'd by ONE 1024-wide activation per (slot, group, kb).
- Causal masks applied by a single broadcast scalar_tensor_tensor on the DVE.
- Softmax denominators (ones-column of the augmented V) inverted with
  reciprocal_approx_fast, broadcast via a K=1 matmul, applied on the DVE.
- w2 is DMA'd in quarters interleaved with the w1 chunk loads so MLP2 never
  waits on HBM.
"""
import os
import sys

for _p in ("/opt/trn_rl_repo", "/root/.axon_site/_ro/trn_rl_repo"):
    if os.path.isdir(_p) and _p not in sys.path:
        sys.path.insert(0, _p)

from contextlib import ExitStack

import ml_dtypes
import numpy as np

import concourse.bass as bass
import concourse.tile as tile
from concourse import mybir
from concourse.bass_utils import run_bass_kernel_spmd
from concourse.masks import make_identity

F32 = mybir.dt.float32
BF16 = mybir.dt.bfloat16
AF = mybir.ActivationFunctionType
OP = mybir.AluOpType

B, T, C, H, D = 4, 1024, 1024, 16, 64
F = 4 * C                       # MLP hidden
NB = T // 128                   # 8 row blocks per batch
NSLOT = 4                       # query blocks per core
KMAX = [8, 6, 4, 2]             # k-blocks computed per slot (max over both cores)
QBLOCKS = [[7, 4, 3, 0], [6, 5, 2, 1]]  # global q-block per slot, by core parity
# (slot, kb) pairs that need a data mask (kb below min over parities: always allow)
MASKED = [(0, 6), (0, 7), (1, 4), (1, 5), (2, 2), (2, 3), (3, 0), (3, 1)]
EPS = 1e-5
NEG = -1e9


def build_nc():
    nc = bass.Bass("TRN2")

    # ---- DRAM I/O ----------------------------------------------------------
    xb = nc.dram_tensor("xb", [T, C], F32, kind="ExternalInput")     # full batch rows
    xq = nc.dram_tensor("xq", [512, C], F32, kind="ExternalInput")   # gathered q rows
    masks = nc.dram_tensor("masks", [8, 128, 128], BF16, kind="ExternalInput")
    wq = nc.dram_tensor("wq", [C, C], BF16, kind="ExternalInput")
    wk = nc.dram_tensor("wk", [C, C], BF16, kind="ExternalInput")
    wv = nc.dram_tensor("wv", [C, C], BF16, kind="ExternalInput")
    wo = nc.dram_tensor("wo", [C, C], BF16, kind="ExternalInput")
    w1 = nc.dram_tensor("w1", [C, F], BF16, kind="ExternalInput")
    w2 = nc.dram_tensor("w2", [F, C], BF16, kind="ExternalInput")
    bq = nc.dram_tensor("bq", [C], F32, kind="ExternalInput")
    bk = nc.dram_tensor("bk", [C], F32, kind="ExternalInput")
    bv = nc.dram_tensor("bv", [C], F32, kind="ExternalInput")
    bo = nc.dram_tensor("bo", [C], F32, kind="ExternalInput")
    b1 = nc.dram_tensor("b1", [F], F32, kind="ExternalInput")
    b2 = nc.dram_tensor("b2", [C], F32, kind="ExternalInput")
    out = nc.dram_tensor("out", [512, C], F32, kind="ExternalOutput")

    with tile.TileContext(nc) as tc, ExitStack() as ctx:
        consts = ctx.enter_context(tc.tile_pool(name="consts", bufs=1))
        small = ctx.enter_context(tc.tile_pool(name="small", bufs=4))

        # ---- constants -----------------------------------------------------
        ones_row = consts.tile([1, 64], BF16, tag="ones_row", name="ones_row")
        nc.vector.memset(ones_row, 1.0)
        eps_col = consts.tile([128, 1], F32, tag="eps", name="eps")
        nc.vector.memset(eps_col, EPS)
        ident_f32 = consts.tile([128, 128], F32, tag="ident", name="ident")
        make_identity(nc, ident_f32)
        one_f32 = consts.tile([1, 1], F32, tag="one1", name="one1")
        nc.vector.memset(one_f32, 1.0)

        def load_cols(dram, nblk, tag):
            t = consts.tile([128, nblk], F32, tag=tag)
            nc.sync.dma_start(out=t, in_=dram.rearrange("(a p) -> p a", p=128))
            return t

        bqc = load_cols(bq, 8, "bqc")
        bkc = load_cols(bk, 8, "bkc")
        b1c = load_cols(b1, 32, "b1c")

        # free-dim biases, broadcast across partitions via DMA
        def load_bcast(dram, tag):
            t = consts.tile([128, C], F32, tag=tag)
            nc.sync.dma_start(
                out=t,
                in_=dram.rearrange("(one c) -> one c", one=1).partition_broadcast(128))
            return t

        BV = load_bcast(bv, "BV")
        BO = load_bcast(bo, "BO")
        B2 = load_bcast(b2, "B2")

        xmid = [consts.tile([128, C], F32, tag=f"xmid{j}", name=f"xmid{j}") for j in range(4)]

        pw1 = ctx.enter_context(tc.tile_pool(name="p_w1", bufs=2))

        def load_w1_chunk(chunk):
            t = pw1.tile([128, 8, C], BF16, tag="w1c", name="w1c")
            for k in range(4):
                nc.sync.dma_start(
                    out=t[:, :, k * 256:(k + 1) * 256],
                    in_=w1[:, chunk * C + k * 256:chunk * C + (k + 1) * 256]
                    .rearrange("(a p) c -> p a c", p=128))
            return t

        att_ctx = ExitStack()
        p_att = att_ctx.enter_context(tc.tile_pool(name="p_att", bufs=1))
        p_w = att_ctx.enter_context(tc.tile_pool(name="p_w", bufs=2))

        mask_sb = p_att.tile([128, 8, 128], BF16, tag="masks", name="masks")
        nc.sync.dma_start(out=mask_sb, in_=masks.rearrange("i p q -> p i q"))

        xq_sb = p_att.tile([128, 4, C], F32, tag="xq", name="xq")
        for _k in range(4):
            nc.sync.dma_start(out=xq_sb[:, _k, :],
                              in_=xq[_k * 128:(_k + 1) * 128, :])

        # attention operands (filled in phase 1)
        qT = p_att.tile([128, 8, 512], BF16, tag="qT", name="qT")
        kT = p_att.tile([128, 8, T], BF16, tag="kT", name="kT")
        vaug = [p_att.tile([128, 16, 65], BF16, tag=f"vaug{t}", name=f"vaug{t}") for t in range(8)]
        # yT_all[hg*64+d, h8, q] = normalized y for head hg*8+h8, dim d.
        # (wo rows are host-permuted to match this head-interleaved layout.)
        yT_all = p_att.tile([128, 8, 512], BF16, tag="yT", name="yT")

        # weight slabs: wv first (needed by the streamed V projection ASAP)
        wv_sb = p_w.tile([128, 8, C], BF16, tag="wslab", name="wslab")
        for _k in range(4):
            nc.sync.dma_start(
                out=wv_sb[:, :, _k * 256:(_k + 1) * 256],
                in_=wv[:, _k * 256:(_k + 1) * 256]
                .rearrange("(a p) c -> p a c", p=128))
        wk_sb = p_w.tile([128, 8, C], BF16, tag="wslab", name="wslab")
        for _k in range(4):
            nc.sync.dma_start(
                out=wk_sb[:, :, _k * 256:(_k + 1) * 256],
                in_=wk[:, _k * 256:(_k + 1) * 256]
                .rearrange("(a p) c -> p a c", p=128))
        wq_sb = p_w.tile([128, 8, C], BF16, tag="wslab", name="wslab")
        for _k in range(4):
            nc.sync.dma_start(
                out=wq_sb[:, :, _k * 256:(_k + 1) * 256],
                in_=wq[:, _k * 256:(_k + 1) * 256]
                .rearrange("(a p) c -> p a c", p=128))

        # ==== phase 1: streamed LN1 -> hT (DMA transpose) -> V/K/Q proj =====
        h1_ctx = ExitStack()
        p_h1 = h1_ctx.enter_context(tc.tile_pool(name="p_h1", bufs=1))
        hT = p_h1.tile([128, 8, 1536], BF16, tag="hT", name="hT")

        ph1s = h1_ctx.enter_context(tc.tile_pool(name="p_h1s", bufs=2))
        ps_qkv = h1_ctx.enter_context(tc.tile_pool(name="ps_qkv", bufs=4, space="PSUM"))

        def ln_block(x_ap, col_off):
            """LN (no gamma/beta) of [128, C] rows -> hT[:, :, col_off:+128]."""
            stats = ph1s.tile([128, 2, 6], F32, tag="ln_stats", name="ln_stats")
            for s in range(2):
                nc.vector.bn_stats(out=stats[:, s, :], in_=x_ap[:, s * 512:(s + 1) * 512])
            mv = ph1s.tile([128, 2], F32, tag="ln_mv", name="ln_mv")
            nc.vector.bn_aggr(out=mv, in_=stats)
            std = ph1s.tile([128, 1], F32, tag="ln_std", name="ln_std")
            nc.scalar.activation(out=std, in_=mv[:, 1:2], func=AF.Sqrt, bias=eps_col)
            rstd = ph1s.tile([128, 1], F32, tag="ln_rstd", name="ln_rstd")
            nc.vector.reciprocal(out=rstd, in_=std)
            h_rows = ph1s.tile([128, C], BF16, tag="h_rows", name="h_rows")
            nc.vector.tensor_scalar(out=h_rows, in0=x_ap, scalar1=mv[:, 0:1],
                                    scalar2=rstd, op0=OP.subtract, op1=OP.mult)
            nc.scalar.dma_start_transpose(out=hT[:, :, col_off:col_off + 128], in_=h_rows)

        def k_proj(nt):
            for co in range(8):
                ps = ps_qkv.tile([128, 512], F32, tag="mm", name="mm")
                for ci in range(8):
                    nc.tensor.matmul(
                        ps, lhsT=wk_sb[:, ci, co * 128:(co + 1) * 128],
                        rhs=hT[:, ci, nt * 512:(nt + 1) * 512],
                        start=(ci == 0), stop=(ci == 7))
                nc.vector.tensor_scalar(
                    out=kT[:, co, nt * 512:(nt + 1) * 512], in0=ps,
                    scalar1=bkc[:, co:co + 1], scalar2=None, op0=OP.add)

        for r in range(NB):
            x_t = ph1s.tile([128, C], F32, tag="x_t", name="x_t")
            for _k in range(2):
                nc.sync.dma_start(
                    out=x_t[:, _k * 512:(_k + 1) * 512],
                    in_=xb[r * 128:(r + 1) * 128, _k * 512:(_k + 1) * 512])
            ln_block(x_t, r * 128)
            # V projection for this row block (+bias), interleaved with ones col
            nc.vector.memset(vaug[r][:, :, 64:65], 1.0)
            for nt in range(2):
                ps = ps_qkv.tile([128, 512], F32, tag="mm", name="mm")
                for ci in range(8):
                    nc.tensor.matmul(
                        ps, lhsT=hT[:, ci, r * 128:(r + 1) * 128],
                        rhs=wv_sb[:, ci, nt * 512:(nt + 1) * 512],
                        start=(ci == 0), stop=(ci == 7))
                nc.vector.scalar_tensor_tensor(
                    out=vaug[r][:, nt * 8:(nt + 1) * 8, 0:64],
                    in0=ps.rearrange("p (h d) -> p h d", d=64),
                    scalar=1.0,
                    in1=BV[:, nt * 512:(nt + 1) * 512]
                        .rearrange("p (h d) -> p h d", d=64),
                    op0=OP.mult, op1=OP.add)
            if r == 3:
                k_proj(0)
            if r == 7:
                k_proj(1)

        for j in range(NSLOT):
            ln_block(xq_sb[:, j, :], 1024 + j * 128)

        # Q^T -> [C, 512]
        for co in range(8):
            ps = ps_qkv.tile([128, 512], F32, tag="mm", name="mm")
            for ci in range(8):
                nc.tensor.matmul(ps, lhsT=wq_sb[:, ci, co * 128:(co + 1) * 128],
                                 rhs=hT[:, ci, 1024:1536], start=(ci == 0), stop=(ci == 7))
            nc.vector.tensor_scalar(out=qT[:, co, :], in0=ps,
                                    scalar1=bqc[:, co:co + 1], scalar2=None,
                                    op0=OP.add)

        wo_sb = p_w.tile([128, 8, C], BF16, tag="wslab", name="wslab")
        for _k in range(4):
            nc.sync.dma_start(
                out=wo_sb[:, :, _k * 256:(_k + 1) * 256],
                in_=wo[:, _k * 256:(_k + 1) * 256]
                .rearrange("(a p) c -> p a c", p=128))

        h1_ctx.close()

        # prefetch the first two w1 chunks during attention
        w1c = [load_w1_chunk(0), load_w1_chunk(1)]

        # ==== phase 2: attention ===========================================
        mask_idx = {sk: i for i, sk in enumerate(MASKED)}
        with tc.tile_pool(name="p_exp", bufs=8) as pexp, \
             tc.tile_pool(name="p_sm", bufs=2) as psm, \
             tc.tile_pool(name="ps_s", bufs=2, space="PSUM") as ps_s, \
             tc.tile_pool(name="ps_y", bufs=1, space="PSUM") as ps_y, \
             tc.tile_pool(name="ps_n", bufs=1, space="PSUM") as ps_n:
            for j in range(NSLOT):
              for hg in range(2):             # head groups of 8
                # scores for all 8 heads of the group: [128, hh*4+hp4, 128]
                # (hh-major so the concurrent row-tiled pair writes distinct
                # PSUM banks)
                expS = [pexp.tile([128, 8, 128], BF16, tag="expS", name="expS")
                        for _ in range(KMAX[j])]
                for kb in range(KMAX[j]):
                    s_ps = ps_s.tile([128, 8, 128], F32, tag="s_ps", name="s_ps")
                    for hp4 in range(4):
                        hp = 4 * hg + hp4
                        for hh in range(2):
                            nc.tensor.matmul(
                                s_ps[:, hh * 4 + hp4, :],
                                lhsT=kT[hh * 64:(hh + 1) * 64, hp,
                                        kb * 128:(kb + 1) * 128],
                                rhs=qT[hh * 64:(hh + 1) * 64, hp,
                                       j * 128:(j + 1) * 128],
                                start=True, stop=True,
                                tile_position=(64 * hh, 0))
                    if (j, kb) in mask_idx:
                        mi = mask_idx[(j, kb)]
                        m_b = mask_sb[:, mi, :].unsqueeze(1) \
                            .broadcast_to([128, 8, 128])
                        sm = psm.tile([128, 8, 128], F32, tag="sm", name="sm")
                        nc.vector.scalar_tensor_tensor(
                            out=sm, in0=s_ps, scalar=0.125, in1=m_b,
                            op0=OP.mult, op1=OP.add)
                        nc.scalar.activation(out=expS[kb], in_=sm, func=AF.Exp)
                    else:
                        nc.scalar.activation(out=expS[kb], in_=s_ps,
                                             func=AF.Exp, scale=0.125)
                # AV for all 8 heads -> one PSUM tile [65, h8, 128]
                # (row 64 = softmax denominator from the vaug ones column)
                yaug8 = ps_y.tile([65, 8, 128], F32, tag="yaug8", name="yaug8")
                for h8 in range(8):
                    h = 8 * hg + h8
                    for kb in range(KMAX[j]):
                        nc.tensor.matmul(
                            yaug8[:, h8, :], lhsT=vaug[kb][:, h, :],
                            rhs=expS[kb][:, (h8 % 2) * 4 + h8 // 2, :],
                            start=(kb == 0), stop=(kb == KMAX[j] - 1))
                # normalization: den row -> transpose to q-partitions ->
                # batched reciprocal -> transpose back -> broadcast -> 1 mul
                den_row = small.tile([1, 8, 128], F32, tag="den_row",
                                     name="den_row", bufs=1)
                nc.vector.tensor_copy(out=den_row, in_=yaug8[64:65, :, :])
                scr = ps_n.tile([128, 8, 128], F32, tag="scr", name="scr")
                dcol = scr[:, 0, 0:8]
                for h8 in range(8):
                    nc.tensor.transpose(dcol[:, h8:h8 + 1],
                                        den_row[0:1, h8, :], one_f32)
                rq = small.tile([128, 8], F32, tag="rq", name="rq", bufs=2)
                nc.vector.reciprocal(out=rq, in_=dcol)
                rT_ps = scr[0:8, 2, :]
                nc.tensor.transpose(rT_ps, rq, ident_f32)
                rT_sb = small.tile([8, 128], BF16, tag="rT_sb", name="rT_sb",
                                   bufs=1)
                nc.vector.tensor_copy(out=rT_sb, in_=rT_ps)
                rrow = small.tile([1, 8, 128], BF16, tag="rrow", name="rrow",
                                  bufs=1)
                nc.sync.dma_start(out=rrow[0:1, :, :], in_=rT_sb[:, :])
                for half in range(2):
                    nc.tensor.matmul(
                        scr[0:64, 4 * half:4 * half + 4, :],
                        lhsT=ones_row,
                        rhs=rrow[0:1, 4 * half:4 * half + 4, :],
                        start=True, stop=True)
                rb8_sb = small.tile([64, 8, 128], BF16, tag="rb8_sb",
                                    name="rb8_sb", bufs=1)
                nc.scalar.mul(rb8_sb, scr[0:64, :, :], 1.0)
                nc.vector.tensor_mul(
                    out=yT_all[hg * 64:(hg + 1) * 64, :,
                               j * 128:(j + 1) * 128],
                    in0=yaug8[0:64, :, :], in1=rb8_sb)

        # ==== phase 3: output projection + residual ========================
        with tc.tile_pool(name="ps_pr", bufs=4, space="PSUM") as ps_pr:
            for j in range(NSLOT):
                for nt in range(2):
                    ps = ps_pr.tile([128, 512], F32, tag="prj", name="prj")
                    for ci in range(8):
                        nc.tensor.matmul(
                            ps, lhsT=yT_all[:, ci, j * 128:(j + 1) * 128],
                            rhs=wo_sb[:, ci, nt * 512:(nt + 1) * 512],
                            start=(ci == 0), stop=(ci == 7))
                    t1 = small.tile([128, 512], F32, tag="prt", name="prt", bufs=2)
                    nc.vector.tensor_add(t1, ps, BO[:, nt * 512:(nt + 1) * 512])
                    nc.vector.tensor_add(
                        xmid[j][:, nt * 512:(nt + 1) * 512], t1,
                        xq_sb[:, j, nt * 512:(nt + 1) * 512])

        att_ctx.close()

        # ==== phase 4: LN2 (DMA transpose) + MLP ===========================
        p_mlp = ctx.enter_context(tc.tile_pool(name="p_mlp", bufs=1))
        h2T = p_mlp.tile([128, 8, 512], BF16, tag="h2T", name="h2T")
        mT = p_mlp.tile([128, 32, 512], BF16, tag="mT", name="mT")

        with tc.tile_pool(name="p_h2s", bufs=2) as ph2s:
            for j in range(NSLOT):
                stats = ph2s.tile([128, 2, 6], F32, tag="ln2_stats", name="ln2_stats")
                for s in range(2):
                    nc.vector.bn_stats(out=stats[:, s, :],
                                       in_=xmid[j][:, s * 512:(s + 1) * 512])
                mv = ph2s.tile([128, 2], F32, tag="ln2_mv", name="ln2_mv")
                nc.vector.bn_aggr(out=mv, in_=stats)
                std = ph2s.tile([128, 1], F32, tag="ln2_std", name="ln2_std")
                nc.scalar.activation(out=std, in_=mv[:, 1:2], func=AF.Sqrt, bias=eps_col)
                rstd = ph2s.tile([128, 1], F32, tag="ln2_rstd", name="ln2_rstd")
                nc.vector.reciprocal(out=rstd, in_=std)
                h2_rows = ph2s.tile([128, C], BF16, tag="h2_rows", name="h2_rows")
                nc.vector.tensor_scalar(out=h2_rows, in0=xmid[j], scalar1=mv[:, 0:1],
                                        scalar2=rstd, op0=OP.subtract, op1=OP.mult)
                nc.scalar.dma_start_transpose(out=h2T[:, :, j * 128:(j + 1) * 128],
                                            in_=h2_rows)

        # w1 chunks + w2 quarters, DMA-interleaved so MLP2 never waits.
        # every 2MB load is 4-way ring-split (one DMA ring only moves
        # ~45-65 GB/s; a monolithic transfer serializes on one ring)
        pw2 = ctx.enter_context(tc.tile_pool(name="p_w2", bufs=4))
        w1c.append(load_w1_chunk(2))
        w2q = []
        for q in range(4):
            t = pw2.tile([128, 8, C], BF16, tag="w2q", name="w2q")
            for k in range(4):
                nc.sync.dma_start(
                    out=t[:, :, k * 256:(k + 1) * 256],
                    in_=w2[q * 1024:(q + 1) * 1024, k * 256:(k + 1) * 256]
                    .rearrange("(a p) c -> p a c", p=128))
            w2q.append(t)
            if q == 0:
                w1c.append(load_w1_chunk(3))

        # ==== phase 5: MLP1 + gelu -> mT ===================================
        with tc.tile_pool(name="ps_m1", bufs=4, space="PSUM") as ps_m1:
            for chunk in range(4):
                for co8 in range(8):
                    co = chunk * 8 + co8
                    ps = ps_m1.tile([128, 512], F32, tag="m1", name="m1")
                    for ci in range(8):
                        nc.tensor.matmul(
                            ps, lhsT=w1c[chunk][:, ci, co8 * 128:(co8 + 1) * 128],
                            rhs=h2T[:, ci, :], start=(ci == 0), stop=(ci == 7))
                    nc.scalar.activation(out=mT[:, co, :], in_=ps, func=AF.Gelu,
                                         bias=b1c[:, co:co + 1])

        # ==== phase 6: MLP2 + residual -> out ==============================
        with tc.tile_pool(name="p_out", bufs=2) as pout, \
             tc.tile_pool(name="ps_m2", bufs=8, space="PSUM") as ps_m2:
            pss = [ps_m2.tile([128, 512], F32, tag="m2", name="m2")
                   for _ in range(8)]
            for q in range(4):
                for j in range(NSLOT):
                    for nt in range(2):
                        ps = pss[j * 2 + nt]
                        for ka in range(8):
                            ki = q * 8 + ka
                            nc.tensor.matmul(
                                ps, lhsT=mT[:, ki, j * 128:(j + 1) * 128],
                                rhs=w2q[q][:, ka, nt * 512:(nt + 1) * 512],
                                start=(ki == 0), stop=(ki == 31))
            for j in range(NSLOT):
                o_sb = pout.tile([128, C], F32, tag="o_sb", name="o_sb")
                for nt in range(2):
                    t1 = small.tile([128, 512], F32, tag="prt", name="ot", bufs=2)
                    nc.vector.tensor_add(t1, pss[j * 2 + nt],
                                         B2[:, nt * 512:(nt + 1) * 512])
                    nc.vector.tensor_add(
                        o_sb[:, nt * 512:(nt + 1) * 512], t1,
                        xmid[j][:, nt * 512:(nt + 1) * 512])
                nc.sync.dma_start(out=out[j * 128:(j + 1) * 128, :], in_=o_sb)

    _split_excess_waits(nc)
    return nc


def _split_excess_waits(nc, max_waits=1):
    """walrus rejects engine instructions with >1 sync wait. Hoist excess
    waits onto standalone EventSemaphore (pure-wait) instructions inserted
    just before the offending instruction on the same engine."""
    counter = 0
    for fn in nc.m.functions:
        for bb in fn.blocks:
            insts = bb.instructions
            i = 0
            while i < len(insts):
                inst = insts[i]
                si = getattr(inst, "sync_info", None)
                if os.environ.get("KEEP_DMA_WAITS") and \
                        type(inst).__name__ == "InstDMACopy":
                    i += 1
                    continue
                if (si is not None and si.on_wait
                        and len(si.on_wait) > max_waits):
                    waits = list(si.on_wait)
                    keep, extra = waits[-max_waits:], waits[:-max_waits]
                    for w in extra:
                        ev = mybir.InstEventSemaphore(
                            name=f"splitwait_{counter}", ins=[], outs=[])
                        counter += 1
                        ev.engine = inst.engine
                        ev.bass_nofuse = True
                        ev.sync_info = mybir.SyncInfo(on_wait=[w], on_update=[])
                        nc.register_instruction(ev)
                        insts.insert(i, ev)
                        i += 1
                    inst.sync_info = mybir.SyncInfo(
                        on_wait=keep, on_update=list(si.on_update))
                i += 1


_NC_CACHE = None


def _get_nc():
    global _NC_CACHE
    if _NC_CACHE is None:
        _NC_CACHE = build_nc()
    return _NC_CACHE


def _permute_wo_rows(wo) -> np.ndarray:
    """Reorder wo rows so slab index a=h8, partition p=hg*64+d maps to
    y channel (hg*8+h8)*64+d (the head-interleaved yT_all layout)."""
    wo = np.asarray(wo, np.float32)
    a = np.arange(C)
    p, blk = a % 128, a // 128          # row index within slab layout
    hg, d = p // 64, p % 64
    src_row = (hg * 8 + blk) * 64 + d
    out = np.empty_like(wo)
    out[a] = wo[src_row]
    return out


def make_masks(parity: int) -> np.ndarray:
    """[8,128,128] additive fp32 mask tiles for the MASKED (slot,kb) pairs."""
    tiles = np.zeros((8, 128, 128), np.float32)
    tri = np.where(np.arange(128)[:, None] <= np.arange(128)[None, :], 0.0, NEG)
    for i, (slot, kb) in enumerate(MASKED):
        g = QBLOCKS[parity][slot]
        if kb < g:
            tiles[i] = 0.0
        elif kb == g:
            tiles[i] = tri.astype(np.float32)
        else:
            tiles[i] = NEG
    return tiles


def make_in_maps(x: np.ndarray, weights: dict) -> list[dict]:
    bf = lambda a: np.ascontiguousarray(np.asarray(a, np.float32)).astype(
        ml_dtypes.bfloat16)
    f32 = lambda a: np.ascontiguousarray(np.asarray(a, np.float32))
    g1 = np.asarray(weights["ln1_g"], np.float64)
    be1 = np.asarray(weights["ln1_b"], np.float64)
    g2 = np.asarray(weights["ln2_g"], np.float64)
    be2 = np.asarray(weights["ln2_b"], np.float64)
    # fold LN gamma into the next matmul's weights, LN beta into its bias
    def fold(wname, bname):
        w = np.asarray(weights[wname], np.float64)
        b = np.asarray(weights[bname], np.float64)
        g, be = (g2, be2) if wname == "w1" else (g1, be1)
        return bf(g[:, None] * w), f32(b + be @ w)
    wq_f, bq_f = fold("wq", "bq")
    wk_f, bk_f = fold("wk", "bk")
    wv_f, bv_f = fold("wv", "bv")
    w1_f, b1_f = fold("w1", "b1")
    shared = {
        "wq": wq_f, "bq": bq_f, "wk": wk_f, "bk": bk_f,
        "wv": wv_f, "bv": bv_f,
        "wo": bf(_permute_wo_rows(weights["wo"])), "bo": f32(weights["bo"]),
        "w1": w1_f, "b1": b1_f,
        "w2": bf(weights["w2"]), "b2": f32(weights["b2"]),
    }
    mask_by_parity = [make_masks(0), make_masks(1)]
    in_maps = []
    for core in range(8):
        b, parity = core // 2, core % 2
        qb = QBLOCKS[parity]
        xqg = np.concatenate([x[b, g * 128:(g + 1) * 128, :] for g in qb], axis=0)
        in_maps.append({
            "xb": f32(x[b]), "xq": f32(xqg),
            "masks": mask_by_parity[parity].astype(ml_dtypes.bfloat16),
            **shared,
        })
    return in_maps


def assemble_out(results: list[dict]) -> np.ndarray:
    out = np.empty((B, T, C), np.float32)
    for core in range(8):
        b, parity = core // 2, core % 2
        o = np.asarray(results[core]["out"], np.float32)
        for j, g in enumerate(QBLOCKS[parity]):
            out[b, g * 128:(g + 1) * 128, :] = o[j * 128:(j + 1) * 128, :]
    return out


def kernel(**inputs) -> np.ndarray:
    x = np.asarray(inputs["x"], np.float32)
    nc = _get_nc()
    in_maps = make_in_maps(x, inputs)
    res = run_bass_kernel_spmd(nc, in_maps, list(range(8)))
    return assemble_out(res.results)


if __name__ == "__main__":
    _get_nc()
    print("built ok")


# revision 42
# speedup vs baseline: 1.0271x; 1.0271x over previous
"""Trainium2 Bass kernel for a dense transformer block (B=4, T=1024, C=1024, H=16).

Sharding: 2 cores per batch element (8 cores / 4 batches). Each core computes
K/V (+LN1) for its full batch but only 4 of the 8 query blocks of 128 rows.
Query blocks are interleaved ({7,4,3,0} on even cores, {6,5,2,1} on odd) so the
causal-attention work is balanced; the compiled program is identical on every
core (SPMD) - per-core behaviour comes only from input data (x slice, gathered
query rows, causal-mask tiles).

v2 layout/scheduling:
- LN gamma/beta folded into the next matmul's weights+bias on the HOST
  (wq' = g*wq, bq' = bq + b@wq, ...), so the device LN is only (x-m)*rstd.
- h transposes done by the DMA XBAR (dma_start_transpose) instead of the PE.
- V projection is streamed per row-block right after that block's LN+transpose,
  K projection in two halves, so the tensor engine is densely busy from the
  start (keeps the HAM clock-gate at full rate).
- Attention scores for all 8 heads of a group land in one PSUM tile
  [128, 2, 4, 128] (hh-major so concurrent row-tiled matmuls hit different
  PSUM banks), exp'd by ONE 1024-wide activation per (slot, group, kb).
- Causal masks applied by a single broadcast scalar_tensor_tensor on the DVE.
- Softmax denominators (ones-column of the augmented V) inverted with
  reciprocal_approx_fast, broadcast via a K=1 matmul, applied on the DVE.
- w2 is DMA'd in quarters interleaved with the w1 chunk loads so MLP2 never
  waits on HBM.
"""
import os
import sys

for _p in ("/opt/trn_rl_repo", "/root/.axon_site/_ro/trn_rl_repo"):
    if os.path.isdir(_p) and _p not in sys.path:
        sys.path.insert(0, _p)

from contextlib import ExitStack

import ml_dtypes
import numpy as np

import concourse.bass as bass
import concourse.tile as tile
from concourse import mybir
from concourse.bass_utils import run_bass_kernel_spmd
from concourse.masks import make_identity

F32 = mybir.dt.float32
BF16 = mybir.dt.bfloat16
AF = mybir.ActivationFunctionType
OP = mybir.AluOpType

B, T, C, H, D = 4, 1024, 1024, 16, 64
F = 4 * C                       # MLP hidden
NB = T // 128                   # 8 row blocks per batch
NSLOT = 4                       # query blocks per core
KMAX = [8, 6, 4, 2]             # k-blocks computed per slot (max over both cores)
QBLOCKS = [[7, 4, 3, 0], [6, 5, 2, 1]]  # global q-block per slot, by core parity
# (slot, kb) pairs that need a data mask (kb below min over parities: always allow)
MASKED = [(0, 6), (0, 7), (1, 4), (1, 5), (2, 2), (2, 3), (3, 0), (3, 1)]
EPS = 1e-5
NEG = -1e9


def build_nc():
    nc = bass.Bass("TRN2")

    # ---- DRAM I/O ----------------------------------------------------------
    xb = nc.dram_tensor("xb", [T, C], F32, kind="ExternalInput")     # full batch rows
    xq = nc.dram_tensor("xq", [512, C], F32, kind="ExternalInput")   # gathered q rows
    masks = nc.dram_tensor("masks", [8, 128, 128], BF16, kind="ExternalInput")
    wq = nc.dram_tensor("wq", [C, C], BF16, kind="ExternalInput")
    wk = nc.dram_tensor("wk", [C, C], BF16, kind="ExternalInput")
    wv = nc.dram_tensor("wv", [C, C], BF16, kind="ExternalInput")
    wo = nc.dram_tensor("wo", [C, C], BF16, kind="ExternalInput")
    w1 = nc.dram_tensor("w1", [C, F], BF16, kind="ExternalInput")
    w2 = nc.dram_tensor("w2", [F, C], BF16, kind="ExternalInput")
    bq = nc.dram_tensor("bq", [C], F32, kind="ExternalInput")
    bk = nc.dram_tensor("bk", [C], F32, kind="ExternalInput")
    bv = nc.dram_tensor("bv", [C], F32, kind="ExternalInput")
    bo = nc.dram_tensor("bo", [C], F32, kind="ExternalInput")
    b1 = nc.dram_tensor("b1", [F], F32, kind="ExternalInput")
    b2 = nc.dram_tensor("b2", [C], F32, kind="ExternalInput")
    out = nc.dram_tensor("out", [512, C], F32, kind="ExternalOutput")

    with tile.TileContext(nc) as tc, ExitStack() as ctx:
        consts = ctx.enter_context(tc.tile_pool(name="consts", bufs=1))
        small = ctx.enter_context(tc.tile_pool(name="small", bufs=4))

        # ---- constants -----------------------------------------------------
        ones_row = consts.tile([1, 64], BF16, tag="ones_row", name="ones_row")
        nc.vector.memset(ones_row, 1.0)
        eps_col = consts.tile([128, 1], F32, tag="eps", name="eps")
        nc.vector.memset(eps_col, EPS)
        ident_f32 = consts.tile([128, 128], F32, tag="ident", name="ident")
        make_identity(nc, ident_f32)
        one_f32 = consts.tile([1, 1], F32, tag="one1", name="one1")
        nc.vector.memset(one_f32, 1.0)

        def load_cols(dram, nblk, tag):
            t = consts.tile([128, nblk], F32, tag=tag)
            nc.sync.dma_start(out=t, in_=dram.rearrange("(a p) -> p a", p=128))
            return t

        bqc = load_cols(bq, 8, "bqc")
        bkc = load_cols(bk, 8, "bkc")
        b1c = load_cols(b1, 32, "b1c")

        # free-dim biases, broadcast across partitions via DMA
        def load_bcast(dram, tag):
            t = consts.tile([128, C], F32, tag=tag)
            nc.sync.dma_start(
                out=t,
                in_=dram.rearrange("(one c) -> one c", one=1).partition_broadcast(128))
            return t

        BV = load_bcast(bv, "BV")
        B2 = load_bcast(b2, "B2")

        xmid = [consts.tile([128, C], F32, tag=f"xmid{j}", name=f"xmid{j}") for j in range(4)]

        pw1 = ctx.enter_context(tc.tile_pool(name="p_w1", bufs=2))

        def load_w1_chunk(chunk):
            t = pw1.tile([128, 8, C], BF16, tag="w1c", name="w1c")
            for k in range(4):
                nc.sync.dma_start(
                    out=t[:, :, k * 256:(k + 1) * 256],
                    in_=w1[:, chunk * C + k * 256:chunk * C + (k + 1) * 256]
                    .rearrange("(a p) c -> p a c", p=128))
            return t

        att_ctx = ExitStack()
        p_att = att_ctx.enter_context(tc.tile_pool(name="p_att", bufs=1))
        p_w = att_ctx.enter_context(tc.tile_pool(name="p_w", bufs=2))

        mask_sb = p_att.tile([128, 8, 128], BF16, tag="masks", name="masks")
        nc.sync.dma_start(out=mask_sb, in_=masks.rearrange("i p q -> p i q"))

        xq_sb = p_att.tile([128, 4, C], F32, tag="xq", name="xq")
        for _k in range(4):
            nc.sync.dma_start(out=xq_sb[:, _k, :],
                              in_=xq[_k * 128:(_k + 1) * 128, :])

        # attention operands (filled in phase 1)
        qT = p_att.tile([128, 8, 512], BF16, tag="qT", name="qT")
        kT = p_att.tile([128, 8, T], BF16, tag="kT", name="kT")
        vaug = [p_att.tile([128, 16, 65], BF16, tag=f"vaug{t}", name=f"vaug{t}") for t in range(8)]
        # yT_all[hg*64+d, h8, q] = normalized y for head hg*8+h8, dim d.
        # (wo rows are host-permuted to match this head-interleaved layout.)
        yT_all = p_att.tile([128, 8, 512], BF16, tag="yT", name="yT")

        # weight slabs: wv first (needed by the streamed V projection ASAP)
        wv_sb = p_w.tile([128, 8, C], BF16, tag="wslab", name="wslab")
        for _k in range(4):
            nc.sync.dma_start(
                out=wv_sb[:, :, _k * 256:(_k + 1) * 256],
                in_=wv[:, _k * 256:(_k + 1) * 256]
                .rearrange("(a p) c -> p a c", p=128))
        wk_sb = p_w.tile([128, 8, C], BF16, tag="wslab", name="wslab")
        for _k in range(4):
            nc.sync.dma_start(
                out=wk_sb[:, :, _k * 256:(_k + 1) * 256],
                in_=wk[:, _k * 256:(_k + 1) * 256]
                .rearrange("(a p) c -> p a c", p=128))
        wq_sb = p_w.tile([128, 8, C], BF16, tag="wslab", name="wslab")
        for _k in range(4):
            nc.sync.dma_start(
                out=wq_sb[:, :, _k * 256:(_k + 1) * 256],
                in_=wq[:, _k * 256:(_k + 1) * 256]
                .rearrange("(a p) c -> p a c", p=128))

        # ==== phase 1: streamed LN1 -> hT (DMA transpose) -> V/K/Q proj =====
        h1_ctx = ExitStack()
        p_h1 = h1_ctx.enter_context(tc.tile_pool(name="p_h1", bufs=1))
        hT = p_h1.tile([128, 8, 1536], BF16, tag="hT", name="hT")

        ph1s = h1_ctx.enter_context(tc.tile_pool(name="p_h1s", bufs=2))
        ps_qkv = h1_ctx.enter_context(tc.tile_pool(name="ps_qkv", bufs=4, space="PSUM"))

        def ln_block(x_ap, col_off):
            """LN (no gamma/beta) of [128, C] rows -> hT[:, :, col_off:+128]."""
            stats = ph1s.tile([128, 2, 6], F32, tag="ln_stats", name="ln_stats")
            for s in range(2):
                nc.vector.bn_stats(out=stats[:, s, :], in_=x_ap[:, s * 512:(s + 1) * 512])
            mv = ph1s.tile([128, 2], F32, tag="ln_mv", name="ln_mv")
            nc.vector.bn_aggr(out=mv, in_=stats)
            std = ph1s.tile([128, 1], F32, tag="ln_std", name="ln_std")
            nc.scalar.activation(out=std, in_=mv[:, 1:2], func=AF.Sqrt, bias=eps_col)
            rstd = ph1s.tile([128, 1], F32, tag="ln_rstd", name="ln_rstd")
            nc.vector.reciprocal(out=rstd, in_=std)
            h_rows = ph1s.tile([128, C], BF16, tag="h_rows", name="h_rows")
            nc.vector.tensor_scalar(out=h_rows, in0=x_ap, scalar1=mv[:, 0:1],
                                    scalar2=rstd, op0=OP.subtract, op1=OP.mult)
            nc.sync.dma_start_transpose(out=hT[:, :, col_off:col_off + 128], in_=h_rows)

        def k_proj(nt):
            for co in range(8):
                ps = ps_qkv.tile([128, 512], F32, tag="mm", name="mm")
                for ci in range(8):
                    nc.tensor.matmul(
                        ps, lhsT=wk_sb[:, ci, co * 128:(co + 1) * 128],
                        rhs=hT[:, ci, nt * 512:(nt + 1) * 512],
                        start=(ci == 0), stop=(ci == 7))
                nc.vector.tensor_scalar(
                    out=kT[:, co, nt * 512:(nt + 1) * 512], in0=ps,
                    scalar1=bkc[:, co:co + 1], scalar2=None, op0=OP.add)

        for r in range(NB):
            x_t = ph1s.tile([128, C], F32, tag="x_t", name="x_t", bufs=3)
            for _k in range(2):
                nc.sync.dma_start(
                    out=x_t[:, _k * 512:(_k + 1) * 512],
                    in_=xb[r * 128:(r + 1) * 128, _k * 512:(_k + 1) * 512])
            ln_block(x_t, r * 128)
            # V projection for this row block (+bias), interleaved with ones col
            nc.vector.memset(vaug[r][:, :, 64:65], 1.0)
            for nt in range(2):
                ps = ps_qkv.tile([128, 512], F32, tag="mm", name="mm")
                for ci in range(8):
                    nc.tensor.matmul(
                        ps, lhsT=hT[:, ci, r * 128:(r + 1) * 128],
                        rhs=wv_sb[:, ci, nt * 512:(nt + 1) * 512],
                        start=(ci == 0), stop=(ci == 7))
                nc.vector.scalar_tensor_tensor(
                    out=vaug[r][:, nt * 8:(nt + 1) * 8, 0:64],
                    in0=ps.rearrange("p (h d) -> p h d", d=64),
                    scalar=1.0,
                    in1=BV[:, nt * 512:(nt + 1) * 512]
                        .rearrange("p (h d) -> p h d", d=64),
                    op0=OP.mult, op1=OP.add)
            if r == 3:
                k_proj(0)
            if r == 7:
                k_proj(1)

        for j in range(NSLOT):
            ln_block(xq_sb[:, j, :], 1024 + j * 128)

        # Q^T -> [C, 512]
        for co in range(8):
            ps = ps_qkv.tile([128, 512], F32, tag="mm", name="mm")
            for ci in range(8):
                nc.tensor.matmul(ps, lhsT=wq_sb[:, ci, co * 128:(co + 1) * 128],
                                 rhs=hT[:, ci, 1024:1536], start=(ci == 0), stop=(ci == 7))
            nc.vector.tensor_scalar(out=qT[:, co, :], in0=ps,
                                    scalar1=bqc[:, co:co + 1], scalar2=None,
                                    op0=OP.add)

        wo_sb = p_w.tile([128, 8, C], BF16, tag="wslab", name="wslab")
        for _k in range(4):
            nc.sync.dma_start(
                out=wo_sb[:, :, _k * 256:(_k + 1) * 256],
                in_=wo[:, _k * 256:(_k + 1) * 256]
                .rearrange("(a p) c -> p a c", p=128))

        h1_ctx.close()

        # prefetch the first two w1 chunks during attention
        w1c = [load_w1_chunk(0), load_w1_chunk(1)]

        # ==== phase 2: attention ===========================================
        mask_idx = {sk: i for i, sk in enumerate(MASKED)}
        with tc.tile_pool(name="p_exp", bufs=8) as pexp, \
             tc.tile_pool(name="p_sm", bufs=2) as psm, \
             tc.tile_pool(name="ps_s", bufs=2, space="PSUM") as ps_s, \
             tc.tile_pool(name="ps_y", bufs=1, space="PSUM") as ps_y, \
             tc.tile_pool(name="ps_n", bufs=1, space="PSUM") as ps_n:
            for j in range(NSLOT):
              for hg in range(2):             # head groups of 8
                # scores for all 8 heads of the group: [128, hh*4+hp4, 128]
                # (hh-major so the concurrent row-tiled pair writes distinct
                # PSUM banks)
                expS = [pexp.tile([128, 8, 128], BF16, tag="expS", name="expS")
                        for _ in range(KMAX[j])]
                for kb in range(KMAX[j]):
                    s_ps = ps_s.tile([128, 8, 128], F32, tag="s_ps", name="s_ps")
                    for hp4 in range(4):
                        hp = 4 * hg + hp4
                        for hh in range(2):
                            nc.tensor.matmul(
                                s_ps[:, hh * 4 + hp4, :],
                                lhsT=kT[hh * 64:(hh + 1) * 64, hp,
                                        kb * 128:(kb + 1) * 128],
                                rhs=qT[hh * 64:(hh + 1) * 64, hp,
                                       j * 128:(j + 1) * 128],
                                start=True, stop=True,
                                tile_position=(64 * hh, 0))
                    if (j, kb) in mask_idx:
                        mi = mask_idx[(j, kb)]
                        m_b = mask_sb[:, mi, :].unsqueeze(1) \
                            .broadcast_to([128, 8, 128])
                        sm = psm.tile([128, 8, 128], F32, tag="sm", name="sm")
                        nc.vector.scalar_tensor_tensor(
                            out=sm, in0=s_ps, scalar=0.125, in1=m_b,
                            op0=OP.mult, op1=OP.add)
                        nc.scalar.activation(out=expS[kb], in_=sm, func=AF.Exp)
                    else:
                        nc.scalar.activation(out=expS[kb], in_=s_ps,
                                             func=AF.Exp, scale=0.125)
                # AV for all 8 heads -> one PSUM tile [65, h8, 128]
                # (row 64 = softmax denominator from the vaug ones column)
                yaug8 = ps_y.tile([65, 8, 128], F32, tag="yaug8", name="yaug8")
                for h8 in range(8):
                    h = 8 * hg + h8
                    for kb in range(KMAX[j]):
                        nc.tensor.matmul(
                            yaug8[:, h8, :], lhsT=vaug[kb][:, h, :],
                            rhs=expS[kb][:, (h8 % 2) * 4 + h8 // 2, :],
                            start=(kb == 0), stop=(kb == KMAX[j] - 1))
                # normalization: den row -> transpose to q-partitions ->
                # batched reciprocal -> transpose back -> broadcast -> 1 mul
                den_row = small.tile([1, 8, 128], F32, tag="den_row",
                                     name="den_row", bufs=1)
                nc.vector.tensor_copy(out=den_row, in_=yaug8[64:65, :, :])
                scr = ps_n.tile([128, 8, 128], F32, tag="scr", name="scr")
                dcol = scr[:, 0, 0:8]
                for h8 in range(8):
                    nc.tensor.transpose(dcol[:, h8:h8 + 1],
                                        den_row[0:1, h8, :], one_f32)
                rq = small.tile([128, 8], F32, tag="rq", name="rq", bufs=2)
                nc.vector.reciprocal(out=rq, in_=dcol)
                rT_ps = scr[0:8, 2, :]
                nc.tensor.transpose(rT_ps, rq, ident_f32)
                rT_sb = small.tile([8, 128], BF16, tag="rT_sb", name="rT_sb",
                                   bufs=1)
                nc.vector.tensor_copy(out=rT_sb, in_=rT_ps)
                rrow = small.tile([1, 8, 128], BF16, tag="rrow", name="rrow",
                                  bufs=1)
                nc.sync.dma_start(out=rrow[0:1, :, :], in_=rT_sb[:, :])
                for half in range(2):
                    nc.tensor.matmul(
                        scr[0:64, 4 * half:4 * half + 4, :],
                        lhsT=ones_row,
                        rhs=rrow[0:1, 4 * half:4 * half + 4, :],
                        start=True, stop=True)
                rb8_sb = small.tile([64, 8, 128], BF16, tag="rb8_sb",
                                    name="rb8_sb", bufs=1)
                nc.scalar.mul(rb8_sb, scr[0:64, :, :], 1.0)
                nc.vector.tensor_mul(
                    out=yT_all[hg * 64:(hg + 1) * 64, :,
                               j * 128:(j + 1) * 128],
                    in0=yaug8[0:64, :, :], in1=rb8_sb)

        # ==== phase 3: output projection + residual ========================
        with tc.tile_pool(name="ps_pr", bufs=4, space="PSUM") as ps_pr:
            for j in range(NSLOT):
                for nt in range(2):
                    ps = ps_pr.tile([128, 512], F32, tag="prj", name="prj")
                    for ci in range(8):
                        nc.tensor.matmul(
                            ps, lhsT=yT_all[:, ci, j * 128:(j + 1) * 128],
                            rhs=wo_sb[:, ci, nt * 512:(nt + 1) * 512],
                            start=(ci == 0), stop=(ci == 7))
                    nc.vector.tensor_add(
                        xmid[j][:, nt * 512:(nt + 1) * 512], ps,
                        xq_sb[:, j, nt * 512:(nt + 1) * 512])

        att_ctx.close()

        # ==== phase 4: LN2 (DMA transpose) + MLP ===========================
        p_mlp = ctx.enter_context(tc.tile_pool(name="p_mlp", bufs=1))
        h2T = p_mlp.tile([128, 8, 512], BF16, tag="h2T", name="h2T")
        mT = p_mlp.tile([128, 32, 512], BF16, tag="mT", name="mT")

        with tc.tile_pool(name="p_h2s", bufs=2) as ph2s:
            for j in range(NSLOT):
                stats = ph2s.tile([128, 2, 6], F32, tag="ln2_stats", name="ln2_stats")
                for s in range(2):
                    nc.vector.bn_stats(out=stats[:, s, :],
                                       in_=xmid[j][:, s * 512:(s + 1) * 512])
                mv = ph2s.tile([128, 2], F32, tag="ln2_mv", name="ln2_mv")
                nc.vector.bn_aggr(out=mv, in_=stats)
                std = ph2s.tile([128, 1], F32, tag="ln2_std", name="ln2_std")
                nc.scalar.activation(out=std, in_=mv[:, 1:2], func=AF.Sqrt, bias=eps_col)
                rstd = ph2s.tile([128, 1], F32, tag="ln2_rstd", name="ln2_rstd")
                nc.vector.reciprocal(out=rstd, in_=std)
                h2_rows = ph2s.tile([128, C], BF16, tag="h2_rows", name="h2_rows")
                nc.vector.tensor_scalar(out=h2_rows, in0=xmid[j], scalar1=mv[:, 0:1],
                                        scalar2=rstd, op0=OP.subtract, op1=OP.mult)
                nc.sync.dma_start_transpose(out=h2T[:, :, j * 128:(j + 1) * 128],
                                            in_=h2_rows)

        # w1 chunks + w2 quarters, DMA-interleaved so MLP2 never waits.
        # every 2MB load is 4-way ring-split (one DMA ring only moves
        # ~45-65 GB/s; a monolithic transfer serializes on one ring)
        pw2 = ctx.enter_context(tc.tile_pool(name="p_w2", bufs=4))
        w1c.append(load_w1_chunk(2))
        w2q = []
        for q in range(4):
            t = pw2.tile([128, 8, C], BF16, tag="w2q", name="w2q")
            for k in range(4):
                nc.sync.dma_start(
                    out=t[:, :, k * 256:(k + 1) * 256],
                    in_=w2[q * 1024:(q + 1) * 1024, k * 256:(k + 1) * 256]
                    .rearrange("(a p) c -> p a c", p=128))
            w2q.append(t)
            if q == 0:
                w1c.append(load_w1_chunk(3))

        # ==== phase 5: MLP1 + gelu -> mT ===================================
        with tc.tile_pool(name="ps_m1", bufs=4, space="PSUM") as ps_m1:
            for chunk in range(4):
                for co8 in range(8):
                    co = chunk * 8 + co8
                    ps = ps_m1.tile([128, 512], F32, tag="m1", name="m1")
                    for ci in range(8):
                        nc.tensor.matmul(
                            ps, lhsT=w1c[chunk][:, ci, co8 * 128:(co8 + 1) * 128],
                            rhs=h2T[:, ci, :], start=(ci == 0), stop=(ci == 7))
                    nc.scalar.activation(out=mT[:, co, :], in_=ps, func=AF.Gelu,
                                         bias=b1c[:, co:co + 1])

        # ==== phase 6: MLP2 + residual -> out ==============================
        with tc.tile_pool(name="p_out", bufs=2) as pout, \
             tc.tile_pool(name="ps_m2", bufs=8, space="PSUM") as ps_m2:
            pss = [ps_m2.tile([128, 512], F32, tag="m2", name="m2")
                   for _ in range(8)]
            for q in range(4):
                for j in range(NSLOT):
                    for nt in range(2):
                        ps = pss[j * 2 + nt]
                        for ka in range(8):
                            ki = q * 8 + ka
                            nc.tensor.matmul(
                                ps, lhsT=mT[:, ki, j * 128:(j + 1) * 128],
                                rhs=w2q[q][:, ka, nt * 512:(nt + 1) * 512],
                                start=(ki == 0), stop=(ki == 31))
            for j in range(NSLOT):
                o_sb = pout.tile([128, C], F32, tag="o_sb", name="o_sb")
                for nt in range(2):
                    t1 = small.tile([128, 512], F32, tag="prt", name="ot", bufs=2)
                    nc.vector.tensor_add(t1, pss[j * 2 + nt],
                                         B2[:, nt * 512:(nt + 1) * 512])
                    nc.vector.tensor_add(
                        o_sb[:, nt * 512:(nt + 1) * 512], t1,
                        xmid[j][:, nt * 512:(nt + 1) * 512])
                nc.sync.dma_start(out=out[j * 128:(j + 1) * 128, :], in_=o_sb)

    _split_excess_waits(nc)
    return nc


def _split_excess_waits(nc, max_waits=1):
    """walrus rejects engine instructions with >1 sync wait. Hoist excess
    waits onto standalone EventSemaphore (pure-wait) instructions inserted
    just before the offending instruction on the same engine."""
    counter = 0
    for fn in nc.m.functions:
        for bb in fn.blocks:
            insts = bb.instructions
            i = 0
            while i < len(insts):
                inst = insts[i]
                si = getattr(inst, "sync_info", None)
                if os.environ.get("KEEP_DMA_WAITS") and \
                        type(inst).__name__ == "InstDMACopy":
                    i += 1
                    continue
                if (si is not None and si.on_wait
                        and len(si.on_wait) > max_waits):
                    waits = list(si.on_wait)
                    keep, extra = waits[-max_waits:], waits[:-max_waits]
                    for w in extra:
                        ev = mybir.InstEventSemaphore(
                            name=f"splitwait_{counter}", ins=[], outs=[])
                        counter += 1
                        ev.engine = inst.engine
                        ev.bass_nofuse = True
                        ev.sync_info = mybir.SyncInfo(on_wait=[w], on_update=[])
                        nc.register_instruction(ev)
                        insts.insert(i, ev)
                        i += 1
                    inst.sync_info = mybir.SyncInfo(
                        on_wait=keep, on_update=list(si.on_update))
                i += 1


_NC_CACHE = None


def _get_nc():
    global _NC_CACHE
    if _NC_CACHE is None:
        _NC_CACHE = build_nc()
    return _NC_CACHE


def _permute_wo_rows(wo) -> np.ndarray:
    """Reorder wo rows so slab index a=h8, partition p=hg*64+d maps to
    y channel (hg*8+h8)*64+d (the head-interleaved yT_all layout)."""
    wo = np.asarray(wo, np.float32)
    a = np.arange(C)
    p, blk = a % 128, a // 128          # row index within slab layout
    hg, d = p // 64, p % 64
    src_row = (hg * 8 + blk) * 64 + d
    out = np.empty_like(wo)
    out[a] = wo[src_row]
    return out


def make_masks(parity: int) -> np.ndarray:
    """[8,128,128] additive fp32 mask tiles for the MASKED (slot,kb) pairs."""
    tiles = np.zeros((8, 128, 128), np.float32)
    tri = np.where(np.arange(128)[:, None] <= np.arange(128)[None, :], 0.0, NEG)
    for i, (slot, kb) in enumerate(MASKED):
        g = QBLOCKS[parity][slot]
        if kb < g:
            tiles[i] = 0.0
        elif kb == g:
            tiles[i] = tri.astype(np.float32)
        else:
            tiles[i] = NEG
    return tiles


def make_in_maps(x: np.ndarray, weights: dict) -> list[dict]:
    bf = lambda a: np.ascontiguousarray(np.asarray(a, np.float32)).astype(
        ml_dtypes.bfloat16)
    f32 = lambda a: np.ascontiguousarray(np.asarray(a, np.float32))
    g1 = np.asarray(weights["ln1_g"], np.float64)
    be1 = np.asarray(weights["ln1_b"], np.float64)
    g2 = np.asarray(weights["ln2_g"], np.float64)
    be2 = np.asarray(weights["ln2_b"], np.float64)
    # fold LN gamma into the next matmul's weights, LN beta into its bias
    def fold(wname, bname):
        w = np.asarray(weights[wname], np.float64)
        b = np.asarray(weights[bname], np.float64)
        g, be = (g2, be2) if wname == "w1" else (g1, be1)
        return bf(g[:, None] * w), f32(b + be @ w)
    wq_f, bq_f = fold("wq", "bq")
    wk_f, bk_f = fold("wk", "bk")
    wv_f, bv_f = fold("wv", "bv")
    w1_f, b1_f = fold("w1", "b1")
    shared = {
        "wq": wq_f, "bq": bq_f, "wk": wk_f, "bk": bk_f,
        "wv": wv_f, "bv": bv_f,
        "wo": bf(_permute_wo_rows(weights["wo"])), "bo": f32(weights["bo"]),
        "w1": w1_f, "b1": b1_f,
        "w2": bf(weights["w2"]), "b2": f32(weights["b2"]),
    }
    mask_by_parity = [make_masks(0), make_masks(1)]
    in_maps = []
    for core in range(8):
        b, parity = core // 2, core % 2
        qb = QBLOCKS[parity]
        xqg = np.concatenate([x[b, g * 128:(g + 1) * 128, :] for g in qb],
                             axis=0) + np.asarray(weights["bo"], np.float64)
        in_maps.append({
            "xb": f32(x[b]), "xq": f32(xqg),
            "masks": mask_by_parity[parity].astype(ml_dtypes.bfloat16),
            **shared,
        })
    return in_maps


def assemble_out(results: list[dict]) -> np.ndarray:
    out = np.empty((B, T, C), np.float32)
    for core in range(8):
        b, parity = core // 2, core % 2
        o = np.asarray(results[core]["out"], np.float32)
        for j, g in enumerate(QBLOCKS[parity]):
            out[b, g * 128:(g + 1) * 128, :] = o[j * 128:(j + 1) * 128, :]
    return out


def kernel(**inputs) -> np.ndarray:
    x = np.asarray(inputs["x"], np.float32)
    nc = _get_nc()
    in_maps = make_in_maps(x, inputs)
    res = run_bass_kernel_spmd(nc, in_maps, list(range(8)))
    return assemble_out(res.results)


if __name__ == "__main__":
    _get_nc()
    print("built ok")
